# revision 13
# baseline (speedup 1.0000x reference)
"""Trainium2 Bass kernel for nn_LocalNetwork (avgpool3d -> 3x LocallyConnected1D -> upsample3d).

Sharding: pure data parallelism — batch 256 split as 32 per core across 8 cores.

Per-core design (32 batches = 4 load groups of 8; conv pairs of 2 groups):

  Loads: X tile [120, 8192], partition p = (dsl, bl) dsl-major so the
  partition ranges [0:64] (dsl 0-7) and [64:120] (dsl 8-14) are clean
  rectangles for both the DMA AP and the compute quadrant rule.  Each
  descriptor covers a full 32KB (h,w) plane (g2/g3) or a 16KB half
  (g0/g1, split for an earlier first pool).  Measured per-SDMA-engine
  HBM-read rate rises from ~14 B/ns at 8KB descriptors to ~19 at 32KB.

  Pools: free-axis halves [120, 4096] -> P2[:, 256h:256h+256] on DVE
  (tensor_reduce is 1x-mode; free-size-bound, so never partition-split).

  Depth stage: the unshared-weight conv along depth is a matmul with a
  banded 1/48 matrix.  Conv-pair chains are [0:96] wide with the dp=4
  rows of all four groups packed into the middle band rows 32:64
  (matmul out base must be 0/32/64): pair0 hosts dp4 of g0+g1 at rows
  32:48, pair1 hosts dp4 of g2+g3 at 48:64; each per-group matmul
  writes the full [32:64] band (half real columns, half zero) and
  accumulates in PSUM, so no garbage rows survive.  DVE elementwise
  cost is free-size-bound, so the extra band is FREE on DVE; it moves
  the dp4 store traffic onto partitions 32:64 whose a-block classes
  4..7/0..3 complement the gA band (a 40-wide store tile doubles
  classes 0,1; this layout spreads all 8 even classes evenly).

  Stores: full-plane [*, 8192] descriptors (32KB, ~25-27 B/ns/engine),
  x3 depth replication = 3 DMAs reading the same U rows.  gA stores on
  the scalar HWDGE queue, gB on sync, dp4 alternating, so both queues
  carry ~equal bytes and loads/stores overlap from ~45us on.

  Weights: [128, 6144] tile (12 x [*,512] blocks), rows 0:32 / 64:96 =
  per-(bl,dp0-3) weights (loaded twice by DMA — no on-chip replication),
  rows 96:128 = dp4 weights; ~25us of identity-matmul replication from
  the v1 kernel removed.
"""

import numpy as np

import concourse.bass as bass
import concourse.mybir as mybir
from concourse import bacc
from concourse.bass_utils import run_bass_kernel_spmd
from concourse.tile import TileContext

F32 = mybir.dt.float32
ADD = mybir.AluOpType.add
MULT = mybir.AluOpType.mult

N_CORES = 8
B = 256
B_CORE = 32          # batches per core
G = 4                # load groups per core
B_GRP = 8            # batches per group
CORE_ELEMS = B_CORE * 15 * 64 * 128  # 3,932,160
BSTRIDE = 15 * 64 * 128              # 122,880
SLICE = 64 * 128                     # 8192 elems = one (h,w) plane = 32KB
HALF = SLICE // 2

# MM layout: one [120, 96] lhsT block per (group-variant g, tap t), at
# cols 96*(3g+t).  Out rows [0:96] of the pair's tap tile: cols 0:32 =
# part1 of gA (nonzero only for g even), 64:96 = part1 of gB (g odd),
# 32:64 = the dp4 band (sub-rows off(g)+bl, off(g) = 8*(g%2)+16*(g//2)).
# The pair accumulates: start on its even group, stop on the odd one,
# so every row is written exactly (zeros where the variant is blank) and
# every matmul uses PE tile position (0,0) size (128,128) — positions
# (0,32)/(0,96) miscompute fp32 matmuls interleaved across tiles.
MM_COLS = 12 * 96


def _pack_consts(w_depth, b_depth, w_lon, b_lon, w_lat, b_lat):
    """Returns (mm [120, MM_COLS] f32, wts [64, 6144] f32)."""
    mm = np.zeros((120, MM_COLS), np.float32)

    def row(dsl, bl):
        return dsl * 8 + bl

    for g in range(4):
        off = 8 * (g % 2) + 16 * (g // 2)
        p1c = 0 if g % 2 == 0 else 64
        for t, shift in ((0, -1), (1, 0), (2, 1)):
            c0 = 96 * (3 * g + t)
            # part1: out col c = bl*4+dp (dp 0..3), tap src = dp+shift
            for bl in range(8):
                for dp in range(4):
                    src = dp + shift
                    if 0 <= src <= 4:
                        for dsl in range(3 * src, 3 * src + 3):
                            mm[row(dsl, bl), c0 + p1c + bl * 4 + dp] = 1.0 / 48.0
                # dp4 band: dn tap = pooled(3), mid = pooled(4), up = none
                if t < 2:
                    for dsl in range(9 + 3 * t, 12 + 3 * t):
                        mm[row(dsl, bl), c0 + 32 + off + bl] = 1.0 / 48.0

    dp = np.arange(4)[:, None, None]
    ho = np.arange(16)[None, :, None]
    wo = np.arange(32)[None, None, :]
    ld = wo * 112 + ho * 7 + (dp + 1)     # depth seq index, dp 0..3
    ll = dp * 544 + ho * 34 + (wo + 1)    # lon
    lt = dp * 576 + wo * 18 + (ho + 1)    # lat
    ho2 = np.arange(16)[:, None]
    wo2 = np.arange(32)[None, :]
    ld4 = wo2 * 112 + ho2 * 7 + 5         # dp=4
    ll4 = 4 * 544 + ho2 * 34 + (wo2 + 1)
    lt4 = 4 * 576 + wo2 * 18 + (ho2 + 1)

    def t1(vec, idx):
        # part1 rows: (bl, dp0..3) x (ho, wo)
        t = np.broadcast_to(np.asarray(vec)[idx][None], (8, 4, 16, 32))
        return t.reshape(32, 512)

    def t2(vec, idx):
        # part2 rows: 32 identical rows of dp4 weights
        t = np.broadcast_to(np.asarray(vec)[idx][None], (32, 16, 32))
        return t.reshape(32, 512)

    cols = []
    for j in range(3):
        cols.append(np.concatenate(
            [t1(np.asarray(w_depth)[:, j], ld), t2(np.asarray(w_depth)[:, j], ld4)], 0))
    cols.append(np.concatenate([t1(b_depth, ld), t2(b_depth, ld4)], 0))
    for j in range(3):
        cols.append(np.concatenate(
            [t1(np.asarray(w_lon)[:, j], ll), t2(np.asarray(w_lon)[:, j], ll4)], 0))
    cols.append(np.concatenate([t1(b_lon, ll), t2(b_lon, ll4)], 0))
    for j in range(3):
        cols.append(np.concatenate(
            [t1(np.asarray(w_lat)[:, j], lt), t2(np.asarray(w_lat)[:, j], lt4)], 0))
    cols.append(np.concatenate([t1(b_lat, lt), t2(b_lat, lt4)], 0))
    wts = np.concatenate(cols, axis=1)  # [64, 6144]
    return np.ascontiguousarray(mm), np.ascontiguousarray(wts, dtype=np.float32)


def build_nc(reps: int = 1) -> bass.Bass:
    nc = bacc.Bacc("TRN2", target_bir_lowering=False, debug=False)
    x = nc.dram_tensor("x", [CORE_ELEMS], F32, kind="ExternalInput")
    mmc = nc.dram_tensor("mm", [120, MM_COLS], F32, kind="ExternalInput")
    wtc = nc.dram_tensor("wts", [64, 6144], F32, kind="ExternalInput")
    y = nc.dram_tensor("y", [CORE_ELEMS], F32, kind="ExternalOutput")


    with TileContext(nc) as tc:
        with (
            tc.tile_pool(name="cpool", bufs=1) as cpool,
            tc.tile_pool(name="inp", bufs=2) as inp,
            tc.tile_pool(name="outp", bufs=2) as outp,
            tc.tile_pool(name="work", bufs=2) as work,
            tc.tile_pool(name="pads", bufs=1) as pads,
            tc.tile_pool(name="p2p", bufs=2) as p2p,
            tc.tile_pool(name="psum", bufs=2, space="PSUM") as psum,
        ):
            MM = cpool.tile([120, MM_COLS], F32)
            WT = cpool.tile([96, 6144], F32)

            w = lambda i: WT[:, i * 512:(i + 1) * 512]
            state = {}

            def load(g, split_free):
                # [0:64] rows (dsl 0-7) on sync q, [64:120] (dsl 8-14) on
                # scalar q; descriptors = 32KB planes (16KB if split_free)
                off = (g % G) * B_GRP * BSTRIDE
                X = inp.tile([120, SLICE], F32, tag="x", name=f"X{g}")
                if split_free:
                    for c in range(2):
                        nc.sync.dma_start(
                            X[0:64, c * HALF:(c + 1) * HALF],
                            bass.AP(x, off + c * HALF,
                                    [[SLICE, 8], [BSTRIDE, 8], [1, HALF]]))
                        nc.scalar.dma_start(
                            X[64:120, c * HALF:(c + 1) * HALF],
                            bass.AP(x, off + 8 * SLICE + c * HALF,
                                    [[SLICE, 7], [BSTRIDE, 8], [1, HALF]]))
                else:
                    nc.sync.dma_start(
                        X[0:64, :],
                        bass.AP(x, off, [[SLICE, 8], [BSTRIDE, 8], [1, SLICE]]))
                    nc.scalar.dma_start(
                        X[64:120, :],
                        bass.AP(x, off + 8 * SLICE,
                                [[SLICE, 7], [BSTRIDE, 8], [1, SLICE]]))
                state[g] = X

            def load_consts():
                nc.sync.dma_start(MM[:], mmc[:])
                nc.scalar.dma_start(WT[0:32, :], wtc[0:32, :])
                nc.sync.dma_start(WT[64:96, :], wtc[0:32, :])
                nc.scalar.dma_start(WT[32:64, :], wtc[32:64, :])

            def pool_half(g, c):
                X = state[g]
                if c == 0:
                    state[("P2", g)] = p2p.tile([120, 512], F32, tag="p2",
                                                name=f"P2_{g}")
                P2 = state[("P2", g)]
                nc.vector.tensor_reduce(
                    P2[:, c * 256:(c + 1) * 256]
                        .rearrange("p (ho wo) -> p ho wo", ho=8),
                    X[:, c * HALF:(c + 1) * HALF]
                        .rearrange("p (ho hs wo ws) -> p ho wo hs ws",
                                   ho=8, hs=4, wo=32, ws=4),
                    mybir.AxisListType.XY, ADD)
                if c == 1:
                    state.pop(g)

            def mm_group(g):
                # one N=512 matmul per tap: while a PSUM accumulation
                # group is open on a bank, issuing another start=True to
                # other columns of the same bank wipes the open partial
                # sums (HW-measured), so never column-split accumulating
                # matmuls
                k, half = divmod(g % G, 2)
                kk = (g // G) * 2 + k
                if half == 0:
                    state[("S", kk)] = tuple(
                        psum.tile([96, 512], F32, tag=f"S{i}", name=f"S{i}_{kk}")
                        for i in range(3))
                S3 = state[("S", kk)]
                P2 = state.pop(("P2", g))
                gg = g % G
                first = (gg % 2 == 0)
                for t, S in enumerate(S3):
                    c0 = 96 * (3 * gg + t)
                    nc.tensor.matmul(S[0:96, :], MM[:, c0:c0 + 96], P2[:],
                                     start=first, stop=not first)

            def conv_store_pair(kk):
                k = kk % 2
                ga, gb = 2 * kk, 2 * kk + 1
                R = 96
                Sdn, S0, Sup = state.pop(("S", kk))
                r = lambda t: t[0:R, :]
                wd0, wd1, wd2, bd = (w(i)[0:R, :] for i in range(4))
                vl0, vl1, vl2, blon = (w(i)[0:R, :] for i in range(4, 8))
                ul0, ul1, ul2, blat = (w(i)[0:R, :] for i in range(8, 12))

                m = work.tile([96, 512], F32, tag="m", name=f"m_{kk}")[0:R, :]
                m2 = work.tile([96, 512], F32, tag="m2", name=f"m2_{kk}")[0:R, :]
                m3 = work.tile([96, 512], F32, tag="m3", name=f"m3_{kk}")[0:R, :]
                nc.vector.tensor_tensor(m, wd0, Sdn[0:R, :], MULT)
                nc.vector.tensor_tensor(m2, wd1, S0[0:R, :], MULT)
                nc.vector.tensor_tensor(m3, wd2, Sup[0:R, :], MULT)
                nc.vector.tensor_tensor(m3, m3, bd, ADD)
                nc.vector.tensor_tensor(m, m, m2, ADD)
                nc.vector.tensor_tensor(m, m, m3, ADD)
                # relu into lon-padded tile Ydp[p, ho*34 + (wo+1)]
                Ydp = pads.tile([96, 544], F32, tag="ydp", name=f"Ydp_{kk}")[0:R, :]
                Ydpv = Ydp.rearrange("p (ho wp) -> p ho wp", ho=16, wp=34)
                nc.gpsimd.memset(Ydpv[:, :, 0], 0)
                nc.gpsimd.memset(Ydpv[:, :, 33], 0)
                nc.vector.tensor_scalar_max(
                    Ydpv[:, :, 1:33],
                    m.rearrange("p (ho wo) -> p ho wo", ho=16), 0.0)

                # lon conv (along wo, free axis)
                r3 = lambda t: t.rearrange("p (ho wo) -> p ho wo", ho=16)
                mv, m2v, m3v = r3(m), r3(m2), r3(m3)
                nc.vector.tensor_tensor(mv, r3(vl0), Ydpv[:, :, 0:32], MULT)
                nc.vector.tensor_tensor(m2v, r3(vl1), Ydpv[:, :, 1:33], MULT)
                nc.vector.tensor_tensor(m3v, r3(vl2), Ydpv[:, :, 2:34], MULT)
                nc.vector.tensor_tensor(m3v, m3v, r3(blon), ADD)
                nc.vector.tensor_tensor(mv, mv, m2v, ADD)
                nc.vector.tensor_tensor(mv, mv, m3v, ADD)
                # relu into lat-padded tile Ylp[p, (ho+1)*32 + wo]
                Ylp = pads.tile([96, 576], F32, tag="ylp", name=f"Ylp_{kk}")[0:R, :]
                nc.gpsimd.memset(Ylp[:, 0:32], 0)
                nc.gpsimd.memset(Ylp[:, 544:576], 0)
                nc.vector.tensor_scalar_max(Ylp[:, 32:544], m, 0.0)

                # lat conv (along ho, free axis; contiguous slices)
                nc.vector.tensor_tensor(m, ul0, Ylp[:, 0:512], MULT)
                nc.vector.tensor_tensor(m2, ul1, Ylp[:, 32:544], MULT)
                nc.vector.tensor_tensor(m3, ul2, Ylp[:, 64:576], MULT)
                nc.vector.tensor_tensor(m3, m3, blat, ADD)
                nc.vector.tensor_tensor(m, m, m2, ADD)
                nc.vector.tensor_tensor(m, m, m3, ADD)

                # upsample: relu + h-expand, then w-expand into U
                A = pads.tile([96, 2048], F32, tag="A", name=f"A_{kk}")[0:R, :]
                mv = m.rearrange("p (ho wo) -> p ho wo", ho=16)
                nc.vector.tensor_scalar_max(
                    A.rearrange("p (ho hs wo) -> p ho hs wo", ho=16, hs=4),
                    mv.unsqueeze(2).broadcast_to([R, 16, 4, 32]), 0.0)
                U = outp.tile([96, SLICE], F32, tag="u", name=f"U_{kk}")
                nc.vector.tensor_scalar_add(
                    U[0:R, :].rearrange("p (h wo ws) -> p h wo ws", h=64, ws=4),
                    A.rearrange("p (h wo) -> p h wo", h=64)
                        .unsqueeze(3).broadcast_to([R, 64, 32, 4]), 0.0)

                # stores: full 32KB planes; gA -> scalar q, gB -> sync q,
                # dp4 band alternates queue by pair
                for di in range(3):
                    nc.scalar.dma_start(
                        bass.AP(y, (ga % G) * B_GRP * BSTRIDE + di * SLICE,
                                [[BSTRIDE, 8], [3 * SLICE, 4], [1, SLICE]]),
                        U[0:32, :])
                    nc.sync.dma_start(
                        bass.AP(y, (gb % G) * B_GRP * BSTRIDE + di * SLICE,
                                [[BSTRIDE, 8], [3 * SLICE, 4], [1, SLICE]]),
                        U[64:96, :])
                    eng = nc.sync if k == 0 else nc.scalar
                    eng.dma_start(
                        bass.AP(y, 2 * k * B_GRP * BSTRIDE + (12 + di) * SLICE,
                                [[B_GRP * BSTRIDE, 2], [BSTRIDE, 8], [1, SLICE]]),
                        U[32 + 16 * k:48 + 16 * k, :])

            # software-pipelined emission; emission order = priority order
            for rr in range(reps):
                b = rr * G
                load(b + 0, split_free=True)
                if rr == 0:
                    load_consts()
                load(b + 1, split_free=True)
                pool_half(b + 0, 0)
                pool_half(b + 0, 1)
                mm_group(b + 0)
                load(b + 2, split_free=False)
                pool_half(b + 1, 0)
                pool_half(b + 1, 1)
                mm_group(b + 1)
                load(b + 3, split_free=False)
                conv_store_pair(rr * 2 + 0)
                pool_half(b + 2, 0)
                pool_half(b + 2, 1)
                mm_group(b + 2)
                pool_half(b + 3, 0)
                pool_half(b + 3, 1)
                mm_group(b + 3)
                conv_store_pair(rr * 2 + 1)

    nc.compile()
    return nc


_NC_CACHE = {}


def _get_nc(reps: int = 1):
    if reps not in _NC_CACHE:
        _NC_CACHE[reps] = build_nc(reps)
    return _NC_CACHE[reps]


def kernel(x, w_depth, b_depth, w_lon, b_lon, w_lat, b_lat, reps: int = 1,
           **run_kwargs):
    mm, wts = _pack_consts(w_depth, b_depth, w_lon, b_lon, w_lat, b_lat)
    xf = np.ascontiguousarray(np.asarray(x), dtype=np.float32).reshape(N_CORES, CORE_ELEMS)
    in_maps = [{"x": xf[c], "mm": mm, "wts": wts} for c in range(N_CORES)]
    nc = _get_nc(reps)
    res = run_bass_kernel_spmd(nc, in_maps, core_ids=list(range(N_CORES)), **run_kwargs)
    out = np.stack([r["y"] for r in res.results], axis=0)
    out = out.reshape(B, 15, 64, 128, 1)
    if run_kwargs:
        kernel.last_results = res
    return out


# revision 14
# speedup vs baseline: 1.1765x; 1.1765x over previous
"""Trainium2 Bass kernel for nn_LocalNetwork (avgpool3d -> 3x LocallyConnected1D -> upsample3d).

Sharding: pure data parallelism — batch 256 split as 32 per core across 8 cores.

Per-core design (32 batches = 4 load groups of 8; conv pairs of 2 groups):
  partition p = (bl, dslice) [8 x 15 = 120], free = (h, w) plane.

  Queue discipline: q1 (sync HWDGE) carries ONLY loads, in free-column
  pieces per group, strictly in consumption order — each group's pieces
  drain the FIFO ahead of the next group's, so the first pool starts at
  ~11us and load completion is never gated on stores.  q10 (scalar
  HWDGE) carries the constants first (its early bandwidth is otherwise
  idle) and then ALL stores in chain-completion order.  Descriptors are
  16KB on both queues: measured per-SDMA-engine rates are ~14 B/ns at
  8KB, ~17 at 16KB loads, ~23 at 16KB stores, and 32KB gains nothing.

  Depth stage: the unshared-weight depth conv is a matmul with a banded
  1/48 matrix.  Conv-pair chains are [0:96] rows: 0:32 = gA x (dp0..3),
  64:96 = gB, 32:64 = the dp4 rows of all four groups (pair0 hosts g0
  at 32:40 + g1 at 40:48, pair1 hosts g2/g3 at 48:64).  Each (group,
  tap) is ONE N=512 matmul with a [120, 96] lhsT; the pair's first
  group opens the PSUM accumulation (start=True), the second closes it.
  Never column-split accumulating matmuls: a start=True to other
  columns of a bank wipes the bank's open partial sums (HW-measured).
  DVE elementwise cost is free-size-bound, so the dp4 band rides the
  pair chains for FREE, and its store partitions 32:64 spread across
  a-block classes that complement the gA band's.

  Weights [96, 6144]: rows 0:32 / 64:96 = per-(bl,dp0-3) weights loaded
  twice by DMA (the v1 identity-matmul replication burned ~25us of PE),
  rows 32:64 = dp4 weights.

  Upsample: relu folded into the h-expand, w-expand per c-half into U
  [96, 4096] tiles so each half's 9 stores (gA/gB/dp4 x 3 depth
  replicas) issue while the other half still expands.
"""

import numpy as np

import concourse.bass as bass
import concourse.mybir as mybir
from concourse import bacc
from concourse.bass_utils import run_bass_kernel_spmd
from concourse.tile import TileContext

F32 = mybir.dt.float32
ADD = mybir.AluOpType.add
MULT = mybir.AluOpType.mult

N_CORES = 8
B = 256
B_CORE = 32          # batches per core
G = 4                # load groups per core
B_GRP = 8            # batches per group
CORE_ELEMS = B_CORE * 15 * 64 * 128  # 3,932,160
BSTRIDE = 15 * 64 * 128              # 122,880
SLICE = 64 * 128                     # 8192 elems = one (h,w) plane = 32KB
HALF = SLICE // 2

# MM: one [120, 96] lhsT block per (group-variant g, tap t) at cols
# 96*(3g+t); rows p = (bl, dsl).  Block cols 0:32 = part1 of gA (nonzero
# only for g even), 64:96 = part1 of gB (g odd), 32:64 = dp4 band rows
# off(g)+bl with off(g) = 8*(g%2) + 16*(g//2).
MM_COLS = 12 * 96


def _pack_consts(w_depth, b_depth, w_lon, b_lon, w_lat, b_lat):
    """Returns (mm [120, MM_COLS] f32, wts [64, 6144] f32)."""
    mm = np.zeros((120, MM_COLS), np.float32)

    def row(dsl, bl):
        return bl * 15 + dsl

    for g in range(4):
        off = 8 * (g % 2) + 16 * (g // 2)
        p1c = 0 if g % 2 == 0 else 64
        for t, shift in ((0, -1), (1, 0), (2, 1)):
            c0 = 96 * (3 * g + t)
            for bl in range(8):
                for dp in range(4):
                    src = dp + shift
                    if 0 <= src <= 4:
                        for dsl in range(3 * src, 3 * src + 3):
                            mm[row(dsl, bl), c0 + p1c + bl * 4 + dp] = 1.0 / 48.0
                # dp4 band: dn tap = pooled(3), mid = pooled(4), up = none
                if t < 2:
                    for dsl in range(9 + 3 * t, 12 + 3 * t):
                        mm[row(dsl, bl), c0 + 32 + off + bl] = 1.0 / 48.0

    dp = np.arange(4)[:, None, None]
    ho = np.arange(16)[None, :, None]
    wo = np.arange(32)[None, None, :]
    ld = wo * 112 + ho * 7 + (dp + 1)     # depth seq index, dp 0..3
    ll = dp * 544 + ho * 34 + (wo + 1)    # lon
    lt = dp * 576 + wo * 18 + (ho + 1)    # lat
    ho2 = np.arange(16)[:, None]
    wo2 = np.arange(32)[None, :]
    ld4 = wo2 * 112 + ho2 * 7 + 5         # dp=4
    ll4 = 4 * 544 + ho2 * 34 + (wo2 + 1)
    lt4 = 4 * 576 + wo2 * 18 + (ho2 + 1)

    def t1(vec, idx):
        t = np.broadcast_to(np.asarray(vec)[idx][None], (8, 4, 16, 32))
        return t.reshape(32, 512)

    def t2(vec, idx):
        t = np.broadcast_to(np.asarray(vec)[idx][None], (32, 16, 32))
        return t.reshape(32, 512)

    cols = []
    for vec, i1, i4 in (
        (np.asarray(w_depth)[:, 0], ld, ld4), (np.asarray(w_depth)[:, 1], ld, ld4),
        (np.asarray(w_depth)[:, 2], ld, ld4), (np.asarray(b_depth), ld, ld4),
        (np.asarray(w_lon)[:, 0], ll, ll4), (np.asarray(w_lon)[:, 1], ll, ll4),
        (np.asarray(w_lon)[:, 2], ll, ll4), (np.asarray(b_lon), ll, ll4),
        (np.asarray(w_lat)[:, 0], lt, lt4), (np.asarray(w_lat)[:, 1], lt, lt4),
        (np.asarray(w_lat)[:, 2], lt, lt4), (np.asarray(b_lat), lt, lt4),
    ):
        cols.append(np.concatenate([t1(vec, i1), t2(vec, i4)], 0))
    wts = np.concatenate(cols, axis=1)  # [64, 6144]
    return np.ascontiguousarray(mm), np.ascontiguousarray(wts, dtype=np.float32)


def build_nc(reps: int = 1) -> bass.Bass:
    nc = bacc.Bacc("TRN2", target_bir_lowering=False, debug=False)
    x = nc.dram_tensor("x", [CORE_ELEMS], F32, kind="ExternalInput")
    mmc = nc.dram_tensor("mm", [120, MM_COLS], F32, kind="ExternalInput")
    wtc = nc.dram_tensor("wts", [64, 6144], F32, kind="ExternalInput")
    y = nc.dram_tensor("y", [CORE_ELEMS], F32, kind="ExternalOutput")

    with TileContext(nc) as tc:
        with (
            tc.tile_pool(name="cpool", bufs=1) as cpool,
            tc.tile_pool(name="inp", bufs=2) as inp,
            tc.tile_pool(name="outp", bufs=2) as outp,
            tc.tile_pool(name="work", bufs=2) as work,
            tc.tile_pool(name="pads", bufs=1) as pads,
            tc.tile_pool(name="p2p", bufs=2) as p2p,
            tc.tile_pool(name="psum", bufs=2, space="PSUM") as psum,
        ):
            MM = cpool.tile([120, MM_COLS], F32)
            WT = cpool.tile([96, 6144], F32)

            w = lambda i: WT[:, i * 512:(i + 1) * 512]
            state = {}

            def load_half(g, c):
                # one [120, 4096] free-half on q1: 120 x 16KB runs
                off = (g % G) * B_GRP * BSTRIDE + c * HALF
                if c == 0:
                    state[g] = inp.tile([120, SLICE], F32, tag="x", name=f"X{g}")
                nc.sync.dma_start(
                    state[g][:, c * HALF:(c + 1) * HALF],
                    bass.AP(x, off, [[BSTRIDE, 8], [SLICE, 15], [1, HALF]]))

            def load_consts():
                nc.scalar.dma_start(MM[:], mmc[:])
                nc.scalar.dma_start(WT[0:32, :], wtc[0:32, :])
                nc.scalar.dma_start(WT[32:64, :], wtc[32:64, :])
                nc.scalar.dma_start(WT[64:96, :], wtc[0:32, :])

            def pool_half(g, c):
                X = state[g]
                if c == 0:
                    state[("P2", g)] = p2p.tile([120, 512], F32, tag="p2",
                                                name=f"P2_{g}")
                P2 = state[("P2", g)]
                nc.vector.tensor_reduce(
                    P2[:, c * 256:(c + 1) * 256]
                        .rearrange("p (ho wo) -> p ho wo", ho=8),
                    X[:, c * HALF:(c + 1) * HALF]
                        .rearrange("p (ho hs wo ws) -> p ho wo hs ws",
                                   ho=8, hs=4, wo=32, ws=4),
                    mybir.AxisListType.XY, ADD)
                if c == 1:
                    state.pop(g)

            def mm_group(g):
                # one N=512 matmul per tap: while a PSUM accumulation
                # group is open on a bank, a start=True to other columns
                # of the same bank wipes the open partial sums, so never
                # column-split accumulating matmuls
                k, half = divmod(g % G, 2)
                kk = (g // G) * 2 + k
                if half == 0:
                    state[("S", kk)] = tuple(
                        psum.tile([96, 512], F32, tag=f"S{i}", name=f"S{i}_{kk}")
                        for i in range(3))
                S3 = state[("S", kk)]
                P2 = state.pop(("P2", g))
                gg = g % G
                first = (gg % 2 == 0)
                for t, S in enumerate(S3):
                    c0 = 96 * (3 * gg + t)
                    nc.tensor.matmul(S[0:96, :], MM[:, c0:c0 + 96], P2[:],
                                     start=first, stop=not first)

            def conv_store_pair(kk):
                k = kk % 2
                ga, gb = 2 * kk, 2 * kk + 1
                Sdn, S0, Sup = state.pop(("S", kk))
                wd0, wd1, wd2, bd = (w(i) for i in range(4))
                vl0, vl1, vl2, blon = (w(i) for i in range(4, 8))
                ul0, ul1, ul2, blat = (w(i) for i in range(8, 12))

                m = work.tile([96, 512], F32, tag="m", name=f"m_{kk}")
                m2 = work.tile([96, 512], F32, tag="m2", name=f"m2_{kk}")
                m3 = work.tile([96, 512], F32, tag="m3", name=f"m3_{kk}")
                nc.vector.tensor_tensor(m[:], wd0, Sdn[:], MULT)
                nc.vector.tensor_tensor(m2[:], wd1, S0[:], MULT)
                nc.vector.tensor_tensor(m3[:], wd2, Sup[:], MULT)
                nc.vector.tensor_tensor(m3[:], m3[:], bd, ADD)
                nc.vector.tensor_tensor(m[:], m[:], m2[:], ADD)
                nc.vector.tensor_tensor(m[:], m[:], m3[:], ADD)
                # relu into lon-padded tile Ydp[p, ho*34 + (wo+1)]
                Ydp = pads.tile([96, 544], F32, tag="ydp", name=f"Ydp_{kk}")
                Ydpv = Ydp[:].rearrange("p (ho wp) -> p ho wp", ho=16, wp=34)
                nc.gpsimd.memset(Ydpv[:, :, 0], 0)
                nc.gpsimd.memset(Ydpv[:, :, 33], 0)
                nc.vector.tensor_scalar_max(
                    Ydpv[:, :, 1:33],
                    m[:].rearrange("p (ho wo) -> p ho wo", ho=16), 0.0)

                # lon conv (along wo, free axis)
                r3 = lambda t: t[:].rearrange("p (ho wo) -> p ho wo", ho=16)
                mv, m2v, m3v = r3(m), r3(m2), r3(m3)
                nc.vector.tensor_tensor(mv, r3(vl0), Ydpv[:, :, 0:32], MULT)
                nc.vector.tensor_tensor(m2v, r3(vl1), Ydpv[:, :, 1:33], MULT)
                nc.vector.tensor_tensor(m3v, r3(vl2), Ydpv[:, :, 2:34], MULT)
                nc.vector.tensor_tensor(m3v, m3v, r3(blon), ADD)
                nc.vector.tensor_tensor(mv, mv, m2v, ADD)
                nc.vector.tensor_tensor(mv, mv, m3v, ADD)
                # relu into lat-padded tile Ylp[p, (ho+1)*32 + wo]
                Ylp = pads.tile([96, 576], F32, tag="ylp", name=f"Ylp_{kk}")
                nc.gpsimd.memset(Ylp[:, 0:32], 0)
                nc.gpsimd.memset(Ylp[:, 544:576], 0)
                nc.vector.tensor_scalar_max(Ylp[:, 32:544], m[:], 0.0)

                # lat conv (along ho, free axis; contiguous slices)
                nc.vector.tensor_tensor(m[:], ul0, Ylp[:, 0:512], MULT)
                nc.vector.tensor_tensor(m2[:], ul1, Ylp[:, 32:544], MULT)
                nc.vector.tensor_tensor(m3[:], ul2, Ylp[:, 64:576], MULT)
                nc.vector.tensor_tensor(m3[:], m3[:], blat, ADD)
                nc.vector.tensor_tensor(m[:], m[:], m2[:], ADD)
                nc.vector.tensor_tensor(m[:], m[:], m3[:], ADD)

                # upsample: relu + h-expand, then per-c w-expand + stores
                A = pads.tile([96, 2048], F32, tag="A", name=f"A_{kk}")
                mv = m[:].rearrange("p (ho wo) -> p ho wo", ho=16)
                nc.vector.tensor_scalar_max(
                    A[:].rearrange("p (ho hs wo) -> p ho hs wo", ho=16, hs=4),
                    mv.unsqueeze(2).broadcast_to([96, 16, 4, 32]), 0.0)
                for c in range(2):
                    U = outp.tile([96, HALF], F32, tag=f"u{c}", name=f"U{c}_{kk}")
                    nc.vector.tensor_scalar_add(
                        U[:].rearrange("p (h wo ws) -> p h wo ws", h=32, ws=4),
                        A[:, c * 1024:(c + 1) * 1024]
                            .rearrange("p (h wo) -> p h wo", h=32)
                            .unsqueeze(3).broadcast_to([96, 32, 32, 4]), 0.0)
                    for di in range(3):
                        off = c * HALF + di * SLICE
                        nc.scalar.dma_start(
                            bass.AP(y, (ga % G) * B_GRP * BSTRIDE + off,
                                    [[BSTRIDE, 8], [3 * SLICE, 4], [1, HALF]]),
                            U[0:32, :])
                        nc.scalar.dma_start(
                            bass.AP(y, (gb % G) * B_GRP * BSTRIDE + off,
                                    [[BSTRIDE, 8], [3 * SLICE, 4], [1, HALF]]),
                            U[64:96, :])
                        nc.scalar.dma_start(
                            bass.AP(y, 2 * k * B_GRP * BSTRIDE + 12 * SLICE + off,
                                    [[B_GRP * BSTRIDE, 2], [BSTRIDE, 8], [1, HALF]]),
                            U[32 + 16 * k:48 + 16 * k, :])

            # software-pipelined emission; emission order = priority order
            for rr in range(reps):
                b = rr * G
                load_half(b + 0, 0)
                load_half(b + 0, 1)
                if rr == 0:
                    load_consts()
                load_half(b + 1, 0)
                load_half(b + 1, 1)
                pool_half(b + 0, 0)
                pool_half(b + 0, 1)
                mm_group(b + 0)
                load_half(b + 2, 0)
                load_half(b + 2, 1)
                pool_half(b + 1, 0)
                pool_half(b + 1, 1)
                mm_group(b + 1)
                load_half(b + 3, 0)
                load_half(b + 3, 1)
                conv_store_pair(rr * 2 + 0)
                pool_half(b + 2, 0)
                pool_half(b + 2, 1)
                mm_group(b + 2)
                pool_half(b + 3, 0)
                pool_half(b + 3, 1)
                mm_group(b + 3)
                conv_store_pair(rr * 2 + 1)

    nc.compile()
    return nc


_NC_CACHE = {}


def _get_nc(reps: int = 1):
    if reps not in _NC_CACHE:
        _NC_CACHE[reps] = build_nc(reps)
    return _NC_CACHE[reps]


def kernel(x, w_depth, b_depth, w_lon, b_lon, w_lat, b_lat, reps: int = 1,
           **run_kwargs):
    mm, wts = _pack_consts(w_depth, b_depth, w_lon, b_lon, w_lat, b_lat)
    xf = np.ascontiguousarray(np.asarray(x), dtype=np.float32).reshape(N_CORES, CORE_ELEMS)
    in_maps = [{"x": xf[c], "mm": mm, "wts": wts} for c in range(N_CORES)]
    nc = _get_nc(reps)
    res = run_bass_kernel_spmd(nc, in_maps, core_ids=list(range(N_CORES)), **run_kwargs)
    out = np.stack([r["y"] for r in res.results], axis=0)
    out = out.reshape(B, 15, 64, 128, 1)
    if run_kwargs:
        kernel.last_results = res
    return out
